# revision 5
# baseline (speedup 1.0000x reference)
"""GAT (4-layer PyG-style GATConv stack) on 8 Trainium2 NeuronCores.

Sharding: destination-node sharding (graph parallel). Nodes are dealt to the
8 cores by id (core = id % 8), then each core's nodes are ordered by
(lo_degree, hi_degree) so that SPMD windows of 128 destination nodes have
near-uniform in-degree; edges are routed to the core owning their
destination and laid out as [128-node window, K slot columns] so the
per-destination softmax + aggregation become free-dim reductions.

Key tricks:
- Per-head basis rotation B_h with B_h[:,0] = a_src[h]: the attention "als"
  term becomes element 0 of each head's 8-dim block, so gather rows are
  exactly 64 f32 = 256B (dma_gather granularity) and no separate als fetch
  exists. Aggregated messages are un-rotated node-side by one matmul per
  window (layers 1-3: blockdiag(B^-1); layer 4: B^-1 @ W4 folded).
- exp(leaky_relu(z)) == max(exp(z), exp(0.2 z)) exactly (two ACT ops).
- Softmax max-subtraction is skipped: e ranges are tiny (verified ~[-3, 15]),
  exp is safe in f32, and alpha = ex/sum(ex) is invariant to the shift.
- dma_gather uses int16 indices; the 50002-row table is addressed through
  two overlapping views (rows [0, 32768) and [N+2-32768, N+2)), with lo/hi
  membership decided purely by owning core (cores 0-4 lo, 5-7 hi) so the
  node ordering stays free. Slot columns are split per window into lo and hi
  column groups; padded slots point at dummy rows whose rotated "als"
  elements are -1e9 (=> exp contributes exactly 0).

Per layer: stage A (sharded node phase: PE transpose + matmul against
host-folded [W @ blockdiag(B) | W @ a_dst]) -> table shard; AllGather the
table; edge phase (gather, z/exp on ACT+DVE, slot reduces, unrotate, bias,
relu).
"""

import numpy as np

import concourse.bass as bass
import concourse.bacc as bacc
import concourse.tile as tile
from concourse import mybir
from concourse import bass_utils
from concourse.masks import make_identity

F32 = mybir.dt.float32
I16 = mybir.dt.int16
AF = mybir.ActivationFunctionType
OP = mybir.AluOpType

NEG_SLOPE = 0.2
NC_CORES = 8
P = 128
LO_CORES = 5          # cores 0..4 -> lo table view, 5..7 -> hi
TAB_VIEW = 32768      # rows addressable per dma_gather view (int16)
MAX_IDX_CALL = 6016   # max indices per dma_gather call (47 slot columns)


# ----------------------------------------------------------------------------
# Host-side preparation
# ----------------------------------------------------------------------------

def _wrap_idx(idx_list):
    """dma_gather index wrap: [128, ceil(n/16)] int16, value (16g+j, s) =
    idx_list[s*16 + j], replicated across the 8 16-partition groups."""
    n = idx_list.shape[0]
    ncols = (n + 15) // 16
    s_idx, p16 = np.meshgrid(np.arange(ncols), np.arange(16), indexing="ij")
    flat = s_idx * 16 + p16
    vals = idx_list[flat.clip(0, n - 1)].astype(np.int16)
    vals[flat >= n] = 0
    w = np.zeros((P, ncols), np.int16)
    for g in range(8):
        w[g * 16:(g + 1) * 16] = vals.T
    return w


def _prep_graph(edge_index, N):
    src = np.asarray(edge_index[0], dtype=np.int64)
    dst = np.asarray(edge_index[1], dtype=np.int64)
    loops = np.arange(N, dtype=np.int64)
    src_all = np.concatenate([src, loops])
    dst_all = np.concatenate([dst, loops])

    assert N % NC_CORES == 0
    nloc = N // NC_CORES
    nw = (nloc + P - 1) // P

    # core assignment by id (fixes lo/hi membership before any ordering)
    core_of = np.arange(N) % NC_CORES
    is_lo_node = core_of < LO_CORES

    lo_cnt = np.bincount(dst_all[is_lo_node[src_all]], minlength=N)
    hi_cnt = np.bincount(dst_all[~is_lo_node[src_all]], minlength=N)
    # self-loops are already included via src_all membership of dst itself

    # per-core ordering: by (lo_cnt desc, hi_cnt desc)
    loc_of = np.empty(N, dtype=np.int64)
    core_nodes = []
    for i in range(NC_CORES):
        nodes_i = np.where(core_of == i)[0]
        o = np.lexsort((-hi_cnt[nodes_i], -lo_cnt[nodes_i]))
        nodes_i = nodes_i[o]
        loc_of[nodes_i] = np.arange(nloc)
        core_nodes.append(nodes_i)

    tp_of = core_of * nloc + loc_of          # table position 0..N-1
    trow_of = tp_of + 1                      # table row (row 0 = dummy lo)
    hi_base = max(0, (N + 2) - TAB_VIEW)     # hi view = tab[hi_base:]

    # window slot widths (shared across cores)
    Klo = np.zeros(nw, dtype=np.int64)
    Khi = np.zeros(nw, dtype=np.int64)
    for i in range(NC_CORES):
        nodes_i = core_nodes[i]
        lc = lo_cnt[nodes_i]
        hc = hi_cnt[nodes_i]
        for w in range(nw):
            lo_m = lc[w * P:(w + 1) * P]
            hi_m = hc[w * P:(w + 1) * P]
            if lo_m.size:
                Klo[w] = max(Klo[w], int(lo_m.max()))
                Khi[w] = max(Khi[w], int(hi_m.max()))

    # per-edge slot assignment: edges sorted by (dst tp, lo/hi, arrival)
    e_tp = tp_of[dst_all]
    e_is_lo = is_lo_node[src_all]
    eorder = np.lexsort((~e_is_lo, e_tp))    # lo edges first within each dst
    tp_s = e_tp[eorder]
    src_s = src_all[eorder]
    islo_s = e_is_lo[eorder]
    starts = np.searchsorted(tp_s, np.arange(N))
    counts_lo = lo_cnt[np.argsort(tp_of, kind="stable")]  # lo_cnt by tp... recompute
    # slot index within the node's lo or hi run
    pos_in_node = np.arange(tp_s.shape[0], dtype=np.int64) - starts[tp_s]
    lo_of_tp = np.zeros(N, dtype=np.int64)
    lo_of_tp[tp_of] = lo_cnt
    k_slot = np.where(islo_s, pos_in_node, pos_in_node - lo_of_tp[tp_s])

    core_e = tp_s // nloc
    loc_e = tp_s % nloc
    w_e = loc_e // P
    p_e = loc_e % P

    # gather calls: per window, lo block split into chunks of <=47 columns,
    # then hi block likewise. Build the call table + index lists.
    dummy_lo = 0
    dummy_hi_local = (N + 1) - hi_base
    calls = []        # list of (window, is_lo, k_start, k_cols)
    for w in range(nw):
        for is_lo, K in ((True, int(Klo[w])), (False, int(Khi[w]))):
            k = 0
            while k < K:
                kc = min(K - k, MAX_IDX_CALL // P)
                calls.append((w, is_lo, k, kc))
                k += kc

    # per-core index lists per call
    idx_vals = np.empty((NC_CORES,), dtype=object)
    edge_core_masks = [core_e == i for i in range(NC_CORES)]
    per_core = []
    for i in range(NC_CORES):
        m = edge_core_masks[i]
        per_core.append((w_e[m], p_e[m], k_slot[m], src_s[m], islo_s[m]))

    wrapped_arrs = [[] for _ in range(NC_CORES)]
    call_meta = []  # (window, is_lo, k_start, kc, flat_offset, ncols)
    flat_off = 0
    # Precompute per-core per-call lists vectorised:
    # build dense slot->index maps per core
    slot_maps = []
    for i in range(NC_CORES):
        wE, pE, kE, sE, lE = per_core[i]
        lo_map = {}
        # dense arrays per window for lo and hi
        lm = [np.full((P, int(Klo[w])), dummy_lo, np.int64) for w in range(nw)]
        hm = [np.full((P, int(Khi[w])), dummy_hi_local, np.int64) for w in range(nw)]
        lo_sel = lE
        trows = trow_of[sE]
        lo_idx = trows            # lo view row == trow (<= 32767 by construction)
        hi_idx = trows - hi_base
        for w in range(nw):
            mw = wE == w
            mwl = mw & lo_sel
            mwh = mw & ~lo_sel
            if lm[w].size:
                lm[w][pE[mwl], kE[mwl]] = lo_idx[mwl]
            if hm[w].size:
                hm[w][pE[mwh], kE[mwh]] = hi_idx[mwh]
        slot_maps.append((lm, hm))

    for (w, is_lo, k0, kc) in calls:
        n_idx = P * kc
        ncols = (n_idx + 15) // 16
        for i in range(NC_CORES):
            lm, hm = slot_maps[i]
            blk = (lm if is_lo else hm)[w][:, k0:k0 + kc]
            # list position j = k*128 + p
            lst = np.ascontiguousarray(blk.T.reshape(-1))
            wrapped_arrs[i].append(_wrap_idx(lst))
        call_meta.append((w, is_lo, k0, kc, flat_off, ncols))
        flat_off += P * ncols

    idx_flat = []
    for i in range(NC_CORES):
        idx_flat.append(np.ascontiguousarray(
            np.concatenate([a.reshape(-1) for a in wrapped_arrs[i]])))
        assert idx_flat[i].shape[0] == flat_off * 1

    sanity = lo_of_tp  # silence linters
    return (core_nodes, nloc, nw, Klo, Khi, call_meta, flat_off,
            idx_flat, hi_base)


def _make_rot(a):
    """B [D,D] with B[:,0] = a, other columns an orthonormal complement."""
    D = a.shape[0]
    na = np.linalg.norm(a)
    assert na > 1e-4, na
    M = np.eye(D, dtype=np.float64)
    M[:, 0] = a
    Q, _ = np.linalg.qr(M)
    B = Q.copy()
    B[:, 0] = a
    Binv = np.linalg.inv(B)
    return B.astype(np.float32), Binv.astype(np.float32)


def _fold_layer(W, a_s, a_d):
    """Wcat = [W @ blockdiag(B_h) | W @ a_d-fold]  ([F, HD+H]), Binv blockdiag."""
    F = W.shape[0]
    H, D = a_s.shape
    Wr = W.reshape(F, H, D).astype(np.float64)
    Bs, Binvs = [], []
    for h in range(H):
        B, Binv = _make_rot(a_s[h].astype(np.float64))
        Bs.append(B)
        Binvs.append(Binv)
    Wrot = np.einsum("fhd,hde->fhe", Wr, np.stack(Bs)).reshape(F, H * D)
    wad = np.einsum("fhd,hd->fh", Wr, a_d.astype(np.float64))
    wcat = np.concatenate([Wrot, wad], axis=1).astype(np.float32)
    binv_blk = np.zeros((H * D, H * D), np.float32)
    for h in range(H):
        binv_blk[h * D:(h + 1) * D, h * D:(h + 1) * D] = Binvs[h]
    return np.ascontiguousarray(wcat), np.ascontiguousarray(binv_blk)


# ----------------------------------------------------------------------------
# Device program
# ----------------------------------------------------------------------------

def _build_program(N, F_in, HD, C, nloc, nw, call_meta, idx_len, hi_base):
    nc = bacc.Bacc(
        "TRN2",
        target_bir_lowering=False,
        debug=False,
        enable_asserts=False,
        num_devices=NC_CORES,
    )
    rg = [list(range(NC_CORES))]
    NTAB = N + 2

    x_d = nc.dram_tensor("x", [nloc, F_in], F32, kind="ExternalInput")
    idx_d = nc.dram_tensor("idx", [idx_len], I16, kind="ExternalInput")
    wcat1_d = nc.dram_tensor("wcat1", [F_in, HD + 8], F32, kind="ExternalInput")
    wcat2_d = nc.dram_tensor("wcat2", [HD, HD + 8], F32, kind="ExternalInput")
    wcat3_d = nc.dram_tensor("wcat3", [HD, HD + 8], F32, kind="ExternalInput")
    wcat4_d = nc.dram_tensor("wcat4", [HD, HD + 1], F32, kind="ExternalInput")
    binv1_d = nc.dram_tensor("binv1", [HD, HD], F32, kind="ExternalInput")
    binv2_d = nc.dram_tensor("binv2", [HD, HD], F32, kind="ExternalInput")
    binv3_d = nc.dram_tensor("binv3", [HD, HD], F32, kind="ExternalInput")
    w4f_d = nc.dram_tensor("w4f", [HD, C], F32, kind="ExternalInput")
    brep1_d = nc.dram_tensor("brep1", [P, HD], F32, kind="ExternalInput")
    brep2_d = nc.dram_tensor("brep2", [P, HD], F32, kind="ExternalInput")
    brep3_d = nc.dram_tensor("brep3", [P, HD], F32, kind="ExternalInput")
    brep4_d = nc.dram_tensor("brep4", [P, C], F32, kind="ExternalInput")
    out_d = nc.dram_tensor("out", [nloc, C], F32, kind="ExternalOutput")

    tabs, shards, aldds, acts = [], [], [], []
    for L in range(4):
        hh = 8 if L < 3 else 1
        tabs.append(nc.dram_tensor(f"tab{L}", [NTAB, HD], F32, kind="Internal",
                                   addr_space="Shared"))
        shards.append(nc.dram_tensor(f"shard{L}", [nloc, HD], F32, kind="Internal"))
        aldds.append(nc.dram_tensor(f"aldd{L}", [nloc, hh], F32, kind="Internal"))
    for L in range(3):
        acts.append(nc.dram_tensor(f"act{L}", [nloc, HD], F32, kind="Internal"))

    # group gather calls per window for the edge phase
    calls_by_w = [[] for _ in range(nw)]
    for (w, is_lo, k0, kc, off, ncols) in call_meta:
        calls_by_w[w].append((is_lo, k0, kc, off, ncols))

    with tile.TileContext(nc) as tc:
        with tc.tile_pool(name="const", bufs=1) as cp, \
             tc.tile_pool(name="sbA", bufs=3) as sa, \
             tc.tile_pool(name="psA", bufs=2, space="PSUM") as pa, \
             tc.tile_pool(name="sbE", bufs=2) as se, \
             tc.tile_pool(name="psE", bufs=2, space="PSUM") as pe:

            ident = cp.tile([P, P], F32)
            make_identity(nc, ident[:])
            wcat_t = []
            for i, (wd, shape) in enumerate(
                [(wcat1_d, [F_in, HD + 8]), (wcat2_d, [HD, HD + 8]),
                 (wcat3_d, [HD, HD + 8]), (wcat4_d, [HD, HD + 1])]
            ):
                t = cp.tile(shape, F32, tag=f"wcat{i}")
                nc.sync.dma_start(out=t[:], in_=wd.ap())
                wcat_t.append(t)
            unrot_t = []
            for i, wd in enumerate([binv1_d, binv2_d, binv3_d, w4f_d]):
                cols = HD if i < 3 else C
                t = cp.tile([HD, cols], F32, tag=f"unrot{i}")
                nc.sync.dma_start(out=t[:], in_=wd.ap())
                unrot_t.append(t)
            brep_t = []
            for i, (bd, cols) in enumerate(
                [(brep1_d, HD), (brep2_d, HD), (brep3_d, HD), (brep4_d, C)]
            ):
                t = cp.tile([P, cols], F32, tag=f"brep{i}")
                nc.sync.dma_start(out=t[:], in_=bd.ap())
                brep_t.append(t)

            for L in range(4):
                fin = F_in if L == 0 else HD
                actsrc = x_d if L == 0 else acts[L - 1]
                ncol = (HD + 8) if L < 3 else (HD + 1)
                hh = 8 if L < 3 else 1
                dd = 8 if L < 3 else HD
                tab, shard, aldd = tabs[L], shards[L], aldds[L]

                # ---- stage A ----
                for w in range(nw):
                    rows = min(P, nloc - w * P)
                    act_t = sa.tile([P, fin], F32, tag="act_t")
                    if rows < P:
                        nc.gpsimd.memset(act_t[:], 0.0)
                    nc.sync.dma_start(
                        out=act_t[:rows, :],
                        in_=actsrc.ap()[w * P: w * P + rows, :],
                    )
                    tp = pa.tile([fin, P], F32, tag="tp")
                    nc.tensor.transpose(out=tp[:], in_=act_t[:], identity=ident[:])
                    actT = sa.tile([fin, P], F32, tag="actT")
                    nc.scalar.copy(out=actT[:], in_=tp[:])
                    hc = pa.tile([P, ncol], F32, tag="hc")
                    nc.tensor.matmul(
                        out=hc[:], lhsT=actT[:], rhs=wcat_t[L][:],
                        start=True, stop=True,
                    )
                    stage = sa.tile([P, HD], F32, tag="stage")
                    aldt = sa.tile([P, hh], F32, tag="aldst")
                    nc.vector.tensor_copy(out=stage[:], in_=hc[:, 0:HD])
                    nc.vector.tensor_copy(out=aldt[:], in_=hc[:, HD:HD + hh])
                    nc.sync.dma_start(
                        out=aldd.ap()[w * P: w * P + rows, :], in_=aldt[:rows, :]
                    )
                    nc.sync.dma_start(
                        out=shard.ap()[w * P: w * P + rows, :], in_=stage[:rows, :]
                    )

                # dummy rows: "als" slots (element 0 of each head block) = -1e9
                dmy = sa.tile([1, HD], F32, tag="dmy")
                nc.gpsimd.memset(dmy[:], 0.0)
                nc.gpsimd.memset(
                    dmy[:].rearrange("p (h d) -> p h d", d=dd)[:, :, 0:1], -1.0e9
                )
                nc.sync.dma_start(out=tab.ap()[0:1, :], in_=dmy[:])
                nc.sync.dma_start(out=tab.ap()[N + 1: N + 2, :], in_=dmy[:])

                # ---- AllGather table ----
                nc.gpsimd.collective_compute(
                    "AllGather",
                    OP.bypass,
                    replica_groups=rg,
                    ins=[shard.ap()],
                    outs=[tab.ap()[1: N + 1, :]],
                )

                # ---- edge phase ----
                for w in range(nw):
                    rows = min(P, nloc - w * P)
                    wcalls = calls_by_w[w]
                    Ktot = sum(kc for (_, _, kc, _, _) in wcalls)
                    if Ktot == 0:
                        continue
                    aldt = se.tile([P, hh], F32, tag="alde")
                    if rows < P:
                        nc.gpsimd.memset(aldt[:], 0.0)
                    nc.sync.dma_start(
                        out=aldt[:rows, :], in_=aldd.ap()[w * P: w * P + rows, :]
                    )
                    G = se.tile([P, Ktot * HD], F32, tag="G")
                    kacc = 0
                    for (is_lo, k0, kc, off, ncols) in wcalls:
                        it = se.tile([P, ncols], I16, tag="idx")
                        nc.sync.dma_start(
                            out=it[:],
                            in_=idx_d.ap()[off: off + P * ncols].rearrange(
                                "(p c) -> p c", c=ncols
                            ),
                        )
                        if N + 2 <= TAB_VIEW:
                            view = tab.ap()
                        elif is_lo:
                            view = tab.ap()[0:TAB_VIEW, :]
                        else:
                            view = tab.ap()[hi_base: NTAB, :]
                        nc.gpsimd.dma_gather(
                            out_ap=G[:, kacc * HD: (kacc + kc) * HD].rearrange(
                                "p (k c) -> p k c", c=HD
                            ),
                            in_ap=view,
                            idxs_ap=it[:],
                            num_idxs=P * kc,
                            num_idxs_reg=P * kc,
                            elem_size=HD,
                            single_packet=False,
                        )
                        kacc += kc
                    K = Ktot
                    Gk = G[:].rearrange("p (k c) -> p k c", c=HD)

                    Z = se.tile([P, hh * K], F32, tag="Z")
                    als3 = (
                        Gk.rearrange("p k (h d) -> p k h d", d=dd)[:, :, :, 0:1]
                        .rearrange("p k h o -> p (h o) k")
                    )
                    ald3 = aldt[:].rearrange("p (h o) -> p h o", o=1).to_broadcast(
                        [P, hh, K]
                    )
                    nc.vector.tensor_tensor(
                        out=Z[:].rearrange("p (h k) -> p h k", k=K),
                        in0=als3, in1=ald3, op=OP.add,
                    )
                    # exp(leaky_relu(z)) == max(exp(z), exp(0.2 z))
                    EXt = se.tile([P, hh * K], F32, tag="EX")
                    nc.scalar.activation(out=EXt[:], in_=Z[:], func=AF.Exp)
                    nc.scalar.activation(
                        out=Z[:], in_=Z[:], func=AF.Exp, scale=NEG_SLOPE
                    )
                    nc.vector.tensor_tensor(
                        out=Z[:], in0=Z[:], in1=EXt[:], op=OP.max
                    )
                    EX = Z

                    s_t = se.tile([P, hh], F32, tag="s")
                    nc.vector.tensor_reduce(
                        out=s_t[:],
                        in_=EX[:].rearrange("p (h k) -> p h k", k=K),
                        axis=mybir.AxisListType.X,
                        op=OP.add,
                    )
                    invs = se.tile([P, hh], F32, tag="invs")
                    nc.vector.reciprocal(out=invs[:], in_=s_t[:])

                    M = se.tile([P, HD * K], F32, tag="M")
                    G4 = Gk.rearrange("p k (h d) -> p h d k", d=dd)
                    EX4 = (
                        EX[:]
                        .rearrange("p (h k) -> p h k", k=K)
                        .rearrange("p h (o k) -> p h o k", o=1)
                        .to_broadcast([P, hh, dd, K])
                    )
                    nc.vector.tensor_tensor(
                        out=M[:].rearrange("p (h d k) -> p h d k", d=dd, k=K),
                        in0=G4, in1=EX4, op=OP.mult,
                    )
                    raw = se.tile([P, HD], F32, tag="raw")
                    nc.vector.tensor_reduce(
                        out=raw[:],
                        in_=M[:].rearrange("p (f k) -> p f k", k=K),
                        axis=mybir.AxisListType.X,
                        op=OP.add,
                    )

                    # node-side: 1/s, un-rotate, bias, relu
                    o1 = se.tile([P, HD], F32, tag="o1")
                    if hh > 1:
                        iv3 = invs[:].rearrange("p (h o) -> p h o", o=1).to_broadcast(
                            [P, hh, dd]
                        )
                        nc.vector.tensor_tensor(
                            out=o1[:].rearrange("p (h d) -> p h d", d=dd),
                            in0=raw[:].rearrange("p (h d) -> p h d", d=dd),
                            in1=iv3, op=OP.mult,
                        )
                    else:
                        nc.vector.tensor_scalar_mul(
                            out=o1[:], in0=raw[:], scalar1=invs[:, 0:1]
                        )
                    tp2 = pe.tile([HD, P], F32, tag="tp2")
                    nc.tensor.transpose(out=tp2[:], in_=o1[:], identity=ident[:])
                    o1T = se.tile([HD, P], F32, tag="o1T")
                    nc.scalar.copy(out=o1T[:], in_=tp2[:])
                    ocols = HD if L < 3 else C
                    op_ = pe.tile([P, ocols], F32, tag="op")
                    nc.tensor.matmul(
                        out=op_[:], lhsT=o1T[:], rhs=unrot_t[L][:],
                        start=True, stop=True,
                    )
                    o2 = se.tile([P, ocols], F32, tag="o2")
                    nc.vector.tensor_tensor(
                        out=o2[:], in0=op_[:], in1=brep_t[L][:], op=OP.add
                    )
                    if L < 3:
                        actn = se.tile([P, HD], F32, tag="actn")
                        nc.scalar.activation(out=actn[:], in_=o2[:], func=AF.Relu)
                        nc.sync.dma_start(
                            out=acts[L].ap()[w * P: w * P + rows, :],
                            in_=actn[:rows, :],
                        )
                    else:
                        nc.sync.dma_start(
                            out=out_d.ap()[w * P: w * P + rows, :],
                            in_=o2[:rows, :],
                        )

    nc.compile()
    return nc


# ----------------------------------------------------------------------------
# Public entry points
# ----------------------------------------------------------------------------

def build_all(x, edge_index, W1, a1s, a1d, b1, W2, a2s, a2d, b2,
              W3, a3s, a3d, b3, W4, a4s, a4d, b4):
    x = np.asarray(x, dtype=np.float32)
    N, F_in = x.shape
    HD = W1.shape[1]
    C = W4.shape[1]

    (core_nodes, nloc, nw, Klo, Khi, call_meta, idx_len, idx_flat,
     hi_base) = _prep_graph(edge_index, N)

    wcat1, binv1 = _fold_layer(np.asarray(W1, np.float32),
                               np.asarray(a1s, np.float32),
                               np.asarray(a1d, np.float32))
    wcat2, binv2 = _fold_layer(np.asarray(W2, np.float32),
                               np.asarray(a2s, np.float32),
                               np.asarray(a2d, np.float32))
    wcat3, binv3 = _fold_layer(np.asarray(W3, np.float32),
                               np.asarray(a3s, np.float32),
                               np.asarray(a3d, np.float32))
    W4f = np.asarray(W4, np.float64)
    was4 = W4f @ np.asarray(a4s, np.float64)[0]
    wad4 = W4f @ np.asarray(a4d, np.float64)[0]
    B4, B4inv = _make_rot(was4)
    wcat4 = np.ascontiguousarray(
        np.concatenate([B4, wad4[:, None]], axis=1).astype(np.float32))
    w4fold = np.ascontiguousarray((B4inv.astype(np.float64) @ W4f).astype(np.float32))

    brep = [np.ascontiguousarray(np.tile(np.asarray(b, np.float32)[None, :], (P, 1)))
            for b in (b1, b2, b3, b4)]

    nc = _build_program(N, F_in, HD, C, nloc, nw, call_meta, idx_len, hi_base)

    in_maps = []
    for i in range(NC_CORES):
        in_maps.append({
            "x": np.ascontiguousarray(x[core_nodes[i]]),
            "idx": idx_flat[i],
            "wcat1": wcat1, "wcat2": wcat2, "wcat3": wcat3, "wcat4": wcat4,
            "binv1": binv1, "binv2": binv2, "binv3": binv3, "w4f": w4fold,
            "brep1": brep[0], "brep2": brep[1], "brep3": brep[2], "brep4": brep[3],
        })

    def assemble(outs):
        full = np.empty((N, C), dtype=np.float32)
        for i in range(NC_CORES):
            full[core_nodes[i]] = outs[i]
        return full

    return nc, in_maps, assemble


def kernel(**inputs):
    nc, in_maps, assemble = build_all(**inputs)
    res = bass_utils.run_bass_kernel_spmd(
        nc, in_maps, core_ids=list(range(NC_CORES))
    )
    return assemble([res.results[i]["out"] for i in range(NC_CORES)])


if __name__ == "__main__":
    import reference
    inputs = {k: np.asarray(v) for k, v in reference.setup_inputs().items()}
    out = kernel(**inputs)
    print(out.shape, out.dtype)


# revision 6
# speedup vs baseline: 3.3389x; 3.3389x over previous
"""GAT (4-layer PyG-style GATConv stack) on 8 Trainium2 NeuronCores.

Sharding: destination-node sharding (graph parallel). Nodes are dealt to the
8 cores by id (core = id % 8), then each core's nodes are ordered by
(lo_degree, hi_degree) so that SPMD windows of 128 destination nodes have
near-uniform in-degree; edges are routed to the core owning their
destination and laid out as [128-node window, K slot columns] so the
per-destination softmax + aggregation become free-dim reductions.

Key tricks:
- Per-head basis rotation B_h with B_h[:,0] = a_src[h]: the attention "als"
  term becomes element 0 of each head's 8-dim block, so gather rows are
  exactly 64 f32 = 256B (dma_gather granularity) and no separate als fetch
  exists. Aggregated messages are un-rotated node-side by one matmul per
  window (layers 1-3: blockdiag(B^-1); layer 4: B^-1 @ W4 folded).
- exp(leaky_relu(z)) == max(exp(z), exp(0.2 z)) exactly (two ACT ops).
- Softmax max-subtraction is skipped: e ranges are tiny (verified ~[-3, 15]),
  exp is safe in f32, and alpha = ex/sum(ex) is invariant to the shift.
- dma_gather uses int16 indices; the 50002-row table is addressed through
  two overlapping views (rows [0, 32768) and [N+2-32768, N+2)), with lo/hi
  membership decided purely by owning core (cores 0-4 lo, 5-7 hi) so the
  node ordering stays free. Slot columns are split per window into lo and hi
  column groups; padded slots point at dummy rows whose rotated "als"
  elements are -1e9 (=> exp contributes exactly 0).

Per layer: stage A (sharded node phase: PE transpose + matmul against
host-folded [W @ blockdiag(B) | W @ a_dst]) -> table shard; AllGather the
table; edge phase (gather, z/exp on ACT+DVE, slot reduces, unrotate, bias,
relu).
"""

import numpy as np

import concourse.bass as bass
import concourse.bacc as bacc
import concourse.tile as tile
from concourse import mybir
from concourse import bass_utils
from concourse.masks import make_identity

F32 = mybir.dt.float32
I16 = mybir.dt.int16
AF = mybir.ActivationFunctionType
OP = mybir.AluOpType

NEG_SLOPE = 0.2
NC_CORES = 8
P = 128
LO_CORES = 5          # cores 0..4 -> lo table view, 5..7 -> hi
TAB_VIEW = 32768      # rows addressable per dma_gather view (int16)
MAX_IDX_CALL = 6016   # max indices per dma_gather call (47 slot columns)


# ----------------------------------------------------------------------------
# Host-side preparation
# ----------------------------------------------------------------------------

def _wrap_idx(idx_list):
    """dma_gather index wrap: [128, ceil(n/16)] int16, value (16g+j, s) =
    idx_list[s*16 + j], replicated across the 8 16-partition groups."""
    n = idx_list.shape[0]
    ncols = (n + 15) // 16
    s_idx, p16 = np.meshgrid(np.arange(ncols), np.arange(16), indexing="ij")
    flat = s_idx * 16 + p16
    vals = idx_list[flat.clip(0, n - 1)].astype(np.int16)
    vals[flat >= n] = 0
    w = np.zeros((P, ncols), np.int16)
    for g in range(8):
        w[g * 16:(g + 1) * 16] = vals.T
    return w


def _prep_graph(edge_index, N):
    src = np.asarray(edge_index[0], dtype=np.int64)
    dst = np.asarray(edge_index[1], dtype=np.int64)
    loops = np.arange(N, dtype=np.int64)
    src_all = np.concatenate([src, loops])
    dst_all = np.concatenate([dst, loops])

    assert N % NC_CORES == 0
    nloc = N // NC_CORES
    nw = (nloc + P - 1) // P

    # core assignment by id (fixes lo/hi membership before any ordering)
    core_of = np.arange(N) % NC_CORES
    is_lo_node = core_of < LO_CORES

    lo_cnt = np.bincount(dst_all[is_lo_node[src_all]], minlength=N)
    hi_cnt = np.bincount(dst_all[~is_lo_node[src_all]], minlength=N)
    # self-loops are already included via src_all membership of dst itself

    # per-core ordering: by (lo_cnt desc, hi_cnt desc)
    loc_of = np.empty(N, dtype=np.int64)
    core_nodes = []
    for i in range(NC_CORES):
        nodes_i = np.where(core_of == i)[0]
        o = np.lexsort((-hi_cnt[nodes_i], -lo_cnt[nodes_i]))
        nodes_i = nodes_i[o]
        loc_of[nodes_i] = np.arange(nloc)
        core_nodes.append(nodes_i)

    tp_of = core_of * nloc + loc_of          # table position 0..N-1
    trow_of = tp_of + 1                      # table row (row 0 = dummy lo)
    hi_base = max(0, (N + 2) - TAB_VIEW)     # hi view = tab[hi_base:]

    # window slot widths (shared across cores)
    Klo = np.zeros(nw, dtype=np.int64)
    Khi = np.zeros(nw, dtype=np.int64)
    for i in range(NC_CORES):
        nodes_i = core_nodes[i]
        lc = lo_cnt[nodes_i]
        hc = hi_cnt[nodes_i]
        for w in range(nw):
            lo_m = lc[w * P:(w + 1) * P]
            hi_m = hc[w * P:(w + 1) * P]
            if lo_m.size:
                Klo[w] = max(Klo[w], int(lo_m.max()))
                Khi[w] = max(Khi[w], int(hi_m.max()))

    # per-edge slot assignment: edges sorted by (dst tp, lo/hi, arrival)
    e_tp = tp_of[dst_all]
    e_is_lo = is_lo_node[src_all]
    eorder = np.lexsort((~e_is_lo, e_tp))    # lo edges first within each dst
    tp_s = e_tp[eorder]
    src_s = src_all[eorder]
    islo_s = e_is_lo[eorder]
    starts = np.searchsorted(tp_s, np.arange(N))
    counts_lo = lo_cnt[np.argsort(tp_of, kind="stable")]  # lo_cnt by tp... recompute
    # slot index within the node's lo or hi run
    pos_in_node = np.arange(tp_s.shape[0], dtype=np.int64) - starts[tp_s]
    lo_of_tp = np.zeros(N, dtype=np.int64)
    lo_of_tp[tp_of] = lo_cnt
    k_slot = np.where(islo_s, pos_in_node, pos_in_node - lo_of_tp[tp_s])

    core_e = tp_s // nloc
    loc_e = tp_s % nloc
    w_e = loc_e // P
    p_e = loc_e % P

    # gather calls: per window, lo block split into chunks of <=47 columns,
    # then hi block likewise. Build the call table + index lists.
    dummy_lo = 0
    dummy_hi_local = (N + 1) - hi_base
    calls = []        # list of (window, is_lo, k_start, k_cols)
    for w in range(nw):
        for is_lo, K in ((True, int(Klo[w])), (False, int(Khi[w]))):
            k = 0
            while k < K:
                kc = min(K - k, MAX_IDX_CALL // P)
                calls.append((w, is_lo, k, kc))
                k += kc

    # per-core index lists per call
    idx_vals = np.empty((NC_CORES,), dtype=object)
    edge_core_masks = [core_e == i for i in range(NC_CORES)]
    per_core = []
    for i in range(NC_CORES):
        m = edge_core_masks[i]
        per_core.append((w_e[m], p_e[m], k_slot[m], src_s[m], islo_s[m]))

    wrapped_arrs = [[] for _ in range(NC_CORES)]
    call_meta = []  # (window, is_lo, k_start, kc, flat_offset, ncols)
    flat_off = 0
    # Precompute per-core per-call lists vectorised:
    # build dense slot->index maps per core
    slot_maps = []
    for i in range(NC_CORES):
        wE, pE, kE, sE, lE = per_core[i]
        lo_map = {}
        # dense arrays per window for lo and hi
        lm = [np.full((P, int(Klo[w])), dummy_lo, np.int64) for w in range(nw)]
        hm = [np.full((P, int(Khi[w])), dummy_hi_local, np.int64) for w in range(nw)]
        lo_sel = lE
        trows = trow_of[sE]
        lo_idx = trows            # lo view row == trow (<= 32767 by construction)
        hi_idx = trows - hi_base
        for w in range(nw):
            mw = wE == w
            mwl = mw & lo_sel
            mwh = mw & ~lo_sel
            if lm[w].size:
                lm[w][pE[mwl], kE[mwl]] = lo_idx[mwl]
            if hm[w].size:
                hm[w][pE[mwh], kE[mwh]] = hi_idx[mwh]
        slot_maps.append((lm, hm))

    for (w, is_lo, k0, kc) in calls:
        n_idx = P * kc
        ncols = (n_idx + 15) // 16
        for i in range(NC_CORES):
            lm, hm = slot_maps[i]
            blk = (lm if is_lo else hm)[w][:, k0:k0 + kc]
            # list position j = k*128 + p
            lst = np.ascontiguousarray(blk.T.reshape(-1))
            wrapped_arrs[i].append(_wrap_idx(lst))
        call_meta.append((w, is_lo, k0, kc, flat_off, ncols))
        flat_off += P * ncols

    idx_flat = []
    for i in range(NC_CORES):
        idx_flat.append(np.ascontiguousarray(
            np.concatenate([a.reshape(-1) for a in wrapped_arrs[i]])))
        assert idx_flat[i].shape[0] == flat_off * 1

    sanity = lo_of_tp  # silence linters
    return (core_nodes, nloc, nw, Klo, Khi, call_meta, flat_off,
            idx_flat, hi_base)


def _make_rot(a):
    """B [D,D] with B[:,0] = a, other columns an orthonormal complement."""
    D = a.shape[0]
    na = np.linalg.norm(a)
    assert na > 1e-4, na
    M = np.eye(D, dtype=np.float64)
    M[:, 0] = a
    Q, _ = np.linalg.qr(M)
    B = Q.copy()
    B[:, 0] = a
    Binv = np.linalg.inv(B)
    return B.astype(np.float32), Binv.astype(np.float32)


def _fold_layer(W, a_s, a_d):
    """Wcat = [W @ blockdiag(B_h) | W @ a_d-fold]  ([F, HD+H]), Binv blockdiag."""
    F = W.shape[0]
    H, D = a_s.shape
    Wr = W.reshape(F, H, D).astype(np.float64)
    Bs, Binvs = [], []
    for h in range(H):
        B, Binv = _make_rot(a_s[h].astype(np.float64))
        Bs.append(B)
        Binvs.append(Binv)
    Wrot = np.einsum("fhd,hde->fhe", Wr, np.stack(Bs)).reshape(F, H * D)
    wad = np.einsum("fhd,hd->fh", Wr, a_d.astype(np.float64))
    wcat = np.concatenate([Wrot, wad], axis=1).astype(np.float32)
    binv_blk = np.zeros((H * D, H * D), np.float32)
    for h in range(H):
        binv_blk[h * D:(h + 1) * D, h * D:(h + 1) * D] = Binvs[h]
    return np.ascontiguousarray(wcat), np.ascontiguousarray(binv_blk)


# ----------------------------------------------------------------------------
# Device program
# ----------------------------------------------------------------------------

def _build_program(N, F_in, HD, C, nloc, nw, call_meta, idx_len, hi_base,
                   repeat=1):
    nc = bacc.Bacc(
        "TRN2",
        target_bir_lowering=False,
        debug=False,
        enable_asserts=False,
        num_devices=NC_CORES,
    )
    rg = [list(range(NC_CORES))]
    NTAB = N + 2

    x_d = nc.dram_tensor("x", [nloc, F_in], F32, kind="ExternalInput")
    idx_d = nc.dram_tensor("idx", [idx_len], I16, kind="ExternalInput")
    wcat1_d = nc.dram_tensor("wcat1", [F_in, HD + 8], F32, kind="ExternalInput")
    wcat2_d = nc.dram_tensor("wcat2", [HD, HD + 8], F32, kind="ExternalInput")
    wcat3_d = nc.dram_tensor("wcat3", [HD, HD + 8], F32, kind="ExternalInput")
    wcat4_d = nc.dram_tensor("wcat4", [HD, HD + 1], F32, kind="ExternalInput")
    binv1_d = nc.dram_tensor("binv1", [HD, HD], F32, kind="ExternalInput")
    binv2_d = nc.dram_tensor("binv2", [HD, HD], F32, kind="ExternalInput")
    binv3_d = nc.dram_tensor("binv3", [HD, HD], F32, kind="ExternalInput")
    w4f_d = nc.dram_tensor("w4f", [HD, C], F32, kind="ExternalInput")
    brep1_d = nc.dram_tensor("brep1", [P, HD], F32, kind="ExternalInput")
    brep2_d = nc.dram_tensor("brep2", [P, HD], F32, kind="ExternalInput")
    brep3_d = nc.dram_tensor("brep3", [P, HD], F32, kind="ExternalInput")
    brep4_d = nc.dram_tensor("brep4", [P, C], F32, kind="ExternalInput")
    out_d = nc.dram_tensor("out", [nloc, C], F32, kind="ExternalOutput")

    tabs, shards, aldds, acts = [], [], [], []
    for L in range(4):
        hh = 8 if L < 3 else 1
        tabs.append(nc.dram_tensor(f"tab{L}", [NTAB, HD], F32, kind="Internal",
                                   addr_space="Shared"))
        shards.append(nc.dram_tensor(f"shard{L}", [nloc, HD], F32, kind="Internal"))
        aldds.append(nc.dram_tensor(f"aldd{L}", [nloc, hh], F32, kind="Internal"))
    for L in range(3):
        acts.append(nc.dram_tensor(f"act{L}", [nloc, HD], F32, kind="Internal"))

    # group gather calls per window for the edge phase
    calls_by_w = [[] for _ in range(nw)]
    for (w, is_lo, k0, kc, off, ncols) in call_meta:
        calls_by_w[w].append((is_lo, k0, kc, off, ncols))

    with tile.TileContext(nc) as tc:
        with tc.tile_pool(name="const", bufs=1) as cp, \
             tc.tile_pool(name="sbA", bufs=3) as sa, \
             tc.tile_pool(name="psA", bufs=2, space="PSUM") as pa, \
             tc.tile_pool(name="sbE", bufs=2) as se, \
             tc.tile_pool(name="psE", bufs=2, space="PSUM") as pe:

            ident = cp.tile([P, P], F32)
            make_identity(nc, ident[:])
            wcat_t = []
            for i, (wd, shape) in enumerate(
                [(wcat1_d, [F_in, HD + 8]), (wcat2_d, [HD, HD + 8]),
                 (wcat3_d, [HD, HD + 8]), (wcat4_d, [HD, HD + 1])]
            ):
                t = cp.tile(shape, F32, tag=f"wcat{i}")
                nc.sync.dma_start(out=t[:], in_=wd.ap())
                wcat_t.append(t)
            unrot_t = []
            for i, wd in enumerate([binv1_d, binv2_d, binv3_d, w4f_d]):
                cols = HD if i < 3 else C
                t = cp.tile([HD, cols], F32, tag=f"unrot{i}")
                nc.sync.dma_start(out=t[:], in_=wd.ap())
                unrot_t.append(t)
            brep_t = []
            for i, (bd, cols) in enumerate(
                [(brep1_d, HD), (brep2_d, HD), (brep3_d, HD), (brep4_d, C)]
            ):
                t = cp.tile([P, cols], F32, tag=f"brep{i}")
                nc.sync.dma_start(out=t[:], in_=bd.ap())
                brep_t.append(t)

            for _rep in range(repeat):
              for L in range(4):
                fin = F_in if L == 0 else HD
                actsrc = x_d if L == 0 else acts[L - 1]
                ncol = (HD + 8) if L < 3 else (HD + 1)
                hh = 8 if L < 3 else 1
                dd = 8 if L < 3 else HD
                tab, shard, aldd = tabs[L], shards[L], aldds[L]

                # ---- stage A ----
                for w in range(nw):
                    rows = min(P, nloc - w * P)
                    act_t = sa.tile([P, fin], F32, tag="act_t")
                    if rows < P:
                        nc.gpsimd.memset(act_t[:], 0.0)
                    nc.sync.dma_start(
                        out=act_t[:rows, :],
                        in_=actsrc.ap()[w * P: w * P + rows, :],
                    )
                    tp = pa.tile([fin, P], F32, tag="tp")
                    nc.tensor.transpose(out=tp[:], in_=act_t[:], identity=ident[:])
                    actT = sa.tile([fin, P], F32, tag="actT")
                    nc.scalar.copy(out=actT[:], in_=tp[:])
                    hc = pa.tile([P, ncol], F32, tag="hc")
                    nc.tensor.matmul(
                        out=hc[:], lhsT=actT[:], rhs=wcat_t[L][:],
                        start=True, stop=True,
                    )
                    stage = sa.tile([P, HD], F32, tag="stage")
                    aldt = sa.tile([P, hh], F32, tag="aldst")
                    nc.vector.tensor_copy(out=stage[:], in_=hc[:, 0:HD])
                    nc.vector.tensor_copy(out=aldt[:], in_=hc[:, HD:HD + hh])
                    nc.sync.dma_start(
                        out=aldd.ap()[w * P: w * P + rows, :], in_=aldt[:rows, :]
                    )
                    nc.sync.dma_start(
                        out=shard.ap()[w * P: w * P + rows, :], in_=stage[:rows, :]
                    )

                # dummy rows: "als" slots (element 0 of each head block) = -1e9
                dmy = sa.tile([1, HD], F32, tag="dmy")
                nc.gpsimd.memset(dmy[:], 0.0)
                nc.gpsimd.memset(
                    dmy[:].rearrange("p (h d) -> p h d", d=dd)[:, :, 0:1], -1.0e9
                )
                nc.sync.dma_start(out=tab.ap()[0:1, :], in_=dmy[:])
                nc.sync.dma_start(out=tab.ap()[N + 1: N + 2, :], in_=dmy[:])

                # ---- AllGather table ----
                nc.gpsimd.collective_compute(
                    "AllGather",
                    OP.bypass,
                    replica_groups=rg,
                    ins=[shard.ap()],
                    outs=[tab.ap()[1: N + 1, :]],
                )

                # ---- edge phase ----
                for w in range(nw):
                    rows = min(P, nloc - w * P)
                    wcalls = calls_by_w[w]
                    Ktot = sum(kc for (_, _, kc, _, _) in wcalls)
                    if Ktot == 0:
                        continue
                    aldt = se.tile([P, hh], F32, tag="alde")
                    if rows < P:
                        nc.gpsimd.memset(aldt[:], 0.0)
                    nc.sync.dma_start(
                        out=aldt[:rows, :], in_=aldd.ap()[w * P: w * P + rows, :]
                    )
                    G = se.tile([P, Ktot * HD], F32, tag="G")
                    kacc = 0
                    for (is_lo, k0, kc, off, ncols) in wcalls:
                        it = se.tile([P, ncols], I16, tag="idx")
                        nc.sync.dma_start(
                            out=it[:],
                            in_=idx_d.ap()[off: off + P * ncols].rearrange(
                                "(p c) -> p c", c=ncols
                            ),
                        )
                        if N + 2 <= TAB_VIEW:
                            view = tab.ap()
                        elif is_lo:
                            view = tab.ap()[0:TAB_VIEW, :]
                        else:
                            view = tab.ap()[hi_base: NTAB, :]
                        nc.gpsimd.dma_gather(
                            out_ap=G[:, kacc * HD: (kacc + kc) * HD].rearrange(
                                "p (k c) -> p k c", c=HD
                            ),
                            in_ap=view,
                            idxs_ap=it[:],
                            num_idxs=P * kc,
                            num_idxs_reg=P * kc,
                            elem_size=HD,
                            single_packet=False,
                        )
                        kacc += kc
                    K = Ktot
                    Gk = G[:].rearrange("p (k c) -> p k c", c=HD)

                    Z = se.tile([P, hh * K], F32, tag="Z")
                    als3 = (
                        Gk.rearrange("p k (h d) -> p k h d", d=dd)[:, :, :, 0:1]
                        .rearrange("p k h o -> p (h o) k")
                    )
                    ald3 = aldt[:].rearrange("p (h o) -> p h o", o=1).to_broadcast(
                        [P, hh, K]
                    )
                    nc.vector.tensor_tensor(
                        out=Z[:].rearrange("p (h k) -> p h k", k=K),
                        in0=als3, in1=ald3, op=OP.add,
                    )
                    # exp(leaky_relu(z)) == max(exp(z), exp(0.2 z))
                    EXt = se.tile([P, hh * K], F32, tag="EX")
                    nc.scalar.activation(out=EXt[:], in_=Z[:], func=AF.Exp)
                    nc.scalar.activation(
                        out=Z[:], in_=Z[:], func=AF.Exp, scale=NEG_SLOPE
                    )
                    nc.vector.tensor_tensor(
                        out=Z[:], in0=Z[:], in1=EXt[:], op=OP.max
                    )
                    EX = Z

                    s_t = se.tile([P, hh], F32, tag="s")
                    nc.vector.tensor_reduce(
                        out=s_t[:],
                        in_=EX[:].rearrange("p (h k) -> p h k", k=K),
                        axis=mybir.AxisListType.X,
                        op=OP.add,
                    )
                    invs = se.tile([P, hh], F32, tag="invs")
                    nc.vector.reciprocal(out=invs[:], in_=s_t[:])

                    M = se.tile([P, HD * K], F32, tag="M")
                    G4 = Gk.rearrange("p k (h d) -> p h d k", d=dd)
                    EX4 = (
                        EX[:]
                        .rearrange("p (h k) -> p h k", k=K)
                        .rearrange("p h (o k) -> p h o k", o=1)
                        .to_broadcast([P, hh, dd, K])
                    )
                    nc.vector.tensor_tensor(
                        out=M[:].rearrange("p (h d k) -> p h d k", d=dd, k=K),
                        in0=G4, in1=EX4, op=OP.mult,
                    )
                    raw = se.tile([P, HD], F32, tag="raw")
                    nc.vector.tensor_reduce(
                        out=raw[:],
                        in_=M[:].rearrange("p (f k) -> p f k", k=K),
                        axis=mybir.AxisListType.X,
                        op=OP.add,
                    )

                    # node-side: 1/s, un-rotate, bias, relu
                    o1 = se.tile([P, HD], F32, tag="o1")
                    if hh > 1:
                        iv3 = invs[:].rearrange("p (h o) -> p h o", o=1).to_broadcast(
                            [P, hh, dd]
                        )
                        nc.vector.tensor_tensor(
                            out=o1[:].rearrange("p (h d) -> p h d", d=dd),
                            in0=raw[:].rearrange("p (h d) -> p h d", d=dd),
                            in1=iv3, op=OP.mult,
                        )
                    else:
                        nc.vector.tensor_scalar_mul(
                            out=o1[:], in0=raw[:], scalar1=invs[:, 0:1]
                        )
                    tp2 = pe.tile([HD, P], F32, tag="tp2")
                    nc.tensor.transpose(out=tp2[:], in_=o1[:], identity=ident[:])
                    o1T = se.tile([HD, P], F32, tag="o1T")
                    nc.scalar.copy(out=o1T[:], in_=tp2[:])
                    ocols = HD if L < 3 else C
                    op_ = pe.tile([P, ocols], F32, tag="op")
                    nc.tensor.matmul(
                        out=op_[:], lhsT=o1T[:], rhs=unrot_t[L][:],
                        start=True, stop=True,
                    )
                    o2 = se.tile([P, ocols], F32, tag="o2")
                    nc.vector.tensor_tensor(
                        out=o2[:], in0=op_[:], in1=brep_t[L][:], op=OP.add
                    )
                    if L < 3:
                        actn = se.tile([P, HD], F32, tag="actn")
                        nc.scalar.activation(out=actn[:], in_=o2[:], func=AF.Relu)
                        nc.sync.dma_start(
                            out=acts[L].ap()[w * P: w * P + rows, :],
                            in_=actn[:rows, :],
                        )
                    else:
                        nc.sync.dma_start(
                            out=out_d.ap()[w * P: w * P + rows, :],
                            in_=o2[:rows, :],
                        )

    nc.compile()
    return nc


# ----------------------------------------------------------------------------
# Public entry points
# ----------------------------------------------------------------------------

def build_all(x, edge_index, W1, a1s, a1d, b1, W2, a2s, a2d, b2,
              W3, a3s, a3d, b3, W4, a4s, a4d, b4):
    x = np.asarray(x, dtype=np.float32)
    N, F_in = x.shape
    HD = W1.shape[1]
    C = W4.shape[1]

    (core_nodes, nloc, nw, Klo, Khi, call_meta, idx_len, idx_flat,
     hi_base) = _prep_graph(edge_index, N)
    import os
    repeat = int(os.environ.get('KREPEAT', '1'))

    wcat1, binv1 = _fold_layer(np.asarray(W1, np.float32),
                               np.asarray(a1s, np.float32),
                               np.asarray(a1d, np.float32))
    wcat2, binv2 = _fold_layer(np.asarray(W2, np.float32),
                               np.asarray(a2s, np.float32),
                               np.asarray(a2d, np.float32))
    wcat3, binv3 = _fold_layer(np.asarray(W3, np.float32),
                               np.asarray(a3s, np.float32),
                               np.asarray(a3d, np.float32))
    W4f = np.asarray(W4, np.float64)
    was4 = W4f @ np.asarray(a4s, np.float64)[0]
    wad4 = W4f @ np.asarray(a4d, np.float64)[0]
    B4, B4inv = _make_rot(was4)
    wcat4 = np.ascontiguousarray(
        np.concatenate([B4, wad4[:, None]], axis=1).astype(np.float32))
    w4fold = np.ascontiguousarray((B4inv.astype(np.float64) @ W4f).astype(np.float32))

    brep = [np.ascontiguousarray(np.tile(np.asarray(b, np.float32)[None, :], (P, 1)))
            for b in (b1, b2, b3, b4)]

    nc = _build_program(N, F_in, HD, C, nloc, nw, call_meta, idx_len, hi_base,
                        repeat=repeat)

    in_maps = []
    for i in range(NC_CORES):
        in_maps.append({
            "x": np.ascontiguousarray(x[core_nodes[i]]),
            "idx": idx_flat[i],
            "wcat1": wcat1, "wcat2": wcat2, "wcat3": wcat3, "wcat4": wcat4,
            "binv1": binv1, "binv2": binv2, "binv3": binv3, "w4f": w4fold,
            "brep1": brep[0], "brep2": brep[1], "brep3": brep[2], "brep4": brep[3],
        })

    def assemble(outs):
        full = np.empty((N, C), dtype=np.float32)
        for i in range(NC_CORES):
            full[core_nodes[i]] = outs[i]
        return full

    return nc, in_maps, assemble


def kernel(**inputs):
    nc, in_maps, assemble = build_all(**inputs)
    res = bass_utils.run_bass_kernel_spmd(
        nc, in_maps, core_ids=list(range(NC_CORES))
    )
    return assemble([res.results[i]["out"] for i in range(NC_CORES)])


if __name__ == "__main__":
    import reference
    inputs = {k: np.asarray(v) for k, v in reference.setup_inputs().items()}
    out = kernel(**inputs)
    print(out.shape, out.dtype)
